# revision 1
# baseline (speedup 1.0000x reference)
"""Trainium2 Bass kernel for a dense transformer block.

Layout strategy: channel-major activations ([d, tokens]) so every linear
layer is a natural PE matmul (contraction dim on partitions, weights in
natural [d_in, d_out] layout as lhsT). Softmax is computed transposed
(S^T = [key, q]) without max-subtraction (scores bounded), with row-sums
obtained from a ones-column appended to V during the A@V matmul.

Sharding over 8 cores, no collectives: core c -> batch b=c//4, query
chunks {j, 7-j} (j=c%4, 256 tokens each). LN1/K/V computed redundantly
for the full batch on each core; causality via per-core mask inputs so
the compiled program is identical on all cores (single-NEFF SPMD).
"""

import numpy as np
import ml_dtypes

# Problem constants (hardcoded per task contract)
B, S, D, H, HS, FF = 2, 2048, 1024, 16, 64, 4096
P = 128
ND = D // P          # 8 d-chunks
NT = S // P          # 16 key chunks
NPAIR = H // 2       # 8 head pairs
QW = 256             # query chunk width
OWN = 2 * QW         # 512 owned query tokens per core
NKC0, NKC1 = 8, 16   # uniform key-chunk counts for q-chunk 0 / 1
NFF = FF // P        # 32
NO = OWN // 512      # 1 (owned tokens fit one 512 slice)
EPS = 1e-5
N_CORES = 8

BF16 = ml_dtypes.bfloat16


def build(nc):
    """Build the single-core SPMD program (identical for all cores)."""
    import concourse.mybir as mybir
    from concourse.tile import TileContext
    from contextlib import ExitStack

    dt = mybir.dt
    f32, bf16 = dt.float32, dt.bfloat16
    Exp = mybir.ActivationFunctionType.Exp
    Gelu = mybir.ActivationFunctionType.Gelu
    Sqrt = mybir.ActivationFunctionType.Sqrt
    Identity = mybir.ActivationFunctionType.Identity

    # ---- I/O ----
    xT_d = nc.dram_tensor("xT", [D, S], bf16, kind="ExternalInput")
    xo_d = nc.dram_tensor("xo", [D, OWN], f32, kind="ExternalInput")
    wq_d = nc.dram_tensor("wq", [D, D], bf16, kind="ExternalInput")
    wk_d = nc.dram_tensor("wk", [D, D], bf16, kind="ExternalInput")
    wv_d = nc.dram_tensor("wv", [D, D], bf16, kind="ExternalInput")
    wp_d = nc.dram_tensor("wp", [D, D], bf16, kind="ExternalInput")
    w1_d = nc.dram_tensor("w1", [D, FF], bf16, kind="ExternalInput")
    w2_d = nc.dram_tensor("w2", [FF, D], bf16, kind="ExternalInput")
    bq_d = nc.dram_tensor("bq", [P, ND], f32, kind="ExternalInput")
    bk_d = nc.dram_tensor("bk", [P, ND], f32, kind="ExternalInput")
    bv_d = nc.dram_tensor("bv", [P, ND], f32, kind="ExternalInput")
    bp_d = nc.dram_tensor("bp", [P, ND], f32, kind="ExternalInput")
    b1_d = nc.dram_tensor("b1", [P, NFF], f32, kind="ExternalInput")
    b2_d = nc.dram_tensor("b2", [P, ND], f32, kind="ExternalInput")
    mk_d = nc.dram_tensor("masks", [NKC1, P, 2 * QW], bf16,
                          kind="ExternalInput")
    out_d = nc.dram_tensor("outT", [D, OWN], f32, kind="ExternalOutput")

    with TileContext(nc) as tc, ExitStack() as top:
        const = top.enter_context(tc.tile_pool(name="const", bufs=1))
        rowp = top.enter_context(tc.tile_pool(name="rows", bufs=1))

        ones_bf = const.tile([P, 1], bf16)
        nc.vector.memset(ones_bf, 1.0)
        ones_f = const.tile([1, P], f32)
        nc.vector.memset(ones_f, 1.0)
        eps_t = const.tile([1, 1], f32)
        nc.vector.memset(eps_t, EPS)
        ones_cf = const.tile([P, 1], f32)
        nc.vector.memset(ones_cf, 1.0)

        bias = {}
        for name, dram, w in (("bq", bq_d, ND), ("bk", bk_d, ND),
                              ("bv", bv_d, ND), ("bp", bp_d, ND),
                              ("b1", b1_d, NFF), ("b2", b2_d, ND)):
            t = const.tile([P, w], f32, tag=f"bias_{name}", name=f"bias_{name}")
            nc.sync.dma_start(out=t, in_=dram[:, :])
            bias[name] = t

        def pool_open(**kw):
            cm = tc.tile_pool(**kw)
            return cm, cm.__enter__()

        def pool_close(*cms):
            for cm in cms:
                cm.__exit__(None, None, None)

        def ln_rows(n, sx_ps, sq_ps):
            """row stats [1, n] from Sigma-x / Sigma-x2 PSUM -> (r_row, s_row).
            Tags shared across phases (sequential use)."""
            mean = rowp.tile([1, n], f32, tag="mean", name="mean")
            nc.scalar.mul(mean, sx_ps, 1.0 / D)
            var = rowp.tile([1, n], f32, tag="var", name="var")
            nc.scalar.mul(var, sq_ps, 1.0 / D)
            msq = rowp.tile([1, n], f32, tag="msq", name="msq")
            nc.vector.tensor_mul(msq, mean, mean)
            nc.vector.tensor_sub(var, var, msq)
            std = rowp.tile([1, n], f32, tag="std", name="std")
            nc.scalar.activation(std, var, Sqrt, bias=eps_t)
            r_row = rowp.tile([1, n], f32, tag="r_row", name="r_row")
            nc.vector.reciprocal_approx_fast(r_row, std)
            s_row = rowp.tile([1, n], f32, tag="s_row", name="s_row")
            nc.vector.tensor_mul(s_row, mean, r_row)
            nc.scalar.mul(s_row, s_row, -1.0)
            r_bf = rowp.tile([1, n], bf16, tag="r_bf", name="r_bf")
            nc.vector.tensor_copy(r_bf, r_row)
            s_bf = rowp.tile([1, n], bf16, tag="s_bf", name="s_bf")
            nc.vector.tensor_copy(s_bf, s_row)
            return r_bf, s_bf

        # ---------- long-lived left-stack pools ----------
        xo_cm, xo_pool = pool_open(name="xo", bufs=1, side="right")
        at_cm, at_pool = pool_open(name="attn", bufs=1)
        xo_t = [xo_pool.tile([P, OWN], f32, tag=f"xo{i}", name=f"xo{i}")
                for i in range(ND)]
        attn = [at_pool.tile([P, OWN], bf16, tag=f"at{p}", name=f"at{p}")
                for p in range(NPAIR)]
        # wp weights (phase E) prefetched during attention; right stack
        wgtE_cm, wpoolE = pool_open(name="wgtE", bufs=1, side="right")
        wp_t = [wpoolE.tile([P, D], bf16, tag=f"wp{i}", name=f"wp{i}")
                for i in range(ND)]

        # hT on the right stack: lives phases A..C only
        hT_cm, hT_pool = pool_open(name="hT", bufs=1, side="right")
        hT = [hT_pool.tile([P, S], bf16, tag=f"h{i}", name=f"h{i}")
              for i in range(ND)]

        # =========== Phase A/B: LN1 over full batch -> hT ===========
        with tc.tile_pool(name="xin", bufs=1, side="right") as x_pool, \
             tc.tile_pool(name="sq", bufs=3, side="right") as sq_pool, \
             tc.tile_pool(name="htm", bufs=3, side="right") as ht_pool, \
             tc.tile_pool(name="bcA", bufs=2, side="right") as bc_pool, \
             tc.tile_pool(name="psA", bufs=2, space="PSUM") as psA:

            xt = [x_pool.tile([P, S], bf16, tag=f"x{i}", name=f"x{i}")
                  for i in range(ND)]
            for s in range(S // 512):
                for i in range(ND):
                    nc.sync.dma_start(
                        out=xt[i][:, 512 * s:512 * s + 512],
                        in_=xT_d[P * i:P * i + P, 512 * s:512 * s + 512])
            for i in range(ND):
                nc.sync.dma_start(out=xo_t[i], in_=xo_d[P * i:P * i + P, :])

            for s in range(S // 512):
                sl = slice(512 * s, 512 * s + 512)
                sx_ps = psA.tile([1, 512], f32, tag="sx")
                sq_ps = psA.tile([1, 512], f32, tag="sq")
                for i in range(ND):
                    sqt = sq_pool.tile([P, 512], bf16, tag="sqt")
                    nc.scalar.square(sqt, xt[i][:, sl])
                    nc.tensor.matmul(sx_ps, ones_bf, xt[i][:, sl],
                                     start=(i == 0), stop=(i == ND - 1))
                    nc.tensor.matmul(sq_ps, ones_bf, sqt,
                                     start=(i == 0), stop=(i == ND - 1))
                r_bf, s_bf = ln_rows(512, sx_ps, sq_ps)
                rb = bc_pool.tile([P, 512], bf16, tag="rb")
                nc.gpsimd.partition_broadcast(rb, r_bf)
                sb = bc_pool.tile([P, 512], bf16, tag="sb")
                nc.gpsimd.partition_broadcast(sb, s_bf)
                for i in range(ND):
                    tmp = ht_pool.tile([P, 512], bf16, tag="htmp")
                    nc.vector.tensor_mul(tmp, xt[i][:, sl], rb)
                    nc.vector.tensor_add(hT[i][:, sl], tmp, sb)

        # ---------- K/V/Q pools (left), live through phase D ----------
        kT_cm, kT_pool = pool_open(name="kT", bufs=1)
        v_cm, v_pool = pool_open(name="v65", bufs=1)
        qT_cm, qT_pool = pool_open(name="qT", bufs=1)
        KT = [kT_pool.tile([P, S], bf16, tag=f"k{p}", name=f"k{p}")
              for p in range(NPAIR)]
        V65 = [v_pool.tile([P, H, HS + 1], bf16, tag=f"v{k}", name=f"v{k}")
               for k in range(NT)]
        QT = [qT_pool.tile([P, OWN], bf16, tag=f"q{p}", name=f"q{p}")
              for p in range(NPAIR)]

        # =========== Phase D: attention (fn; called inside C scope) =====
        # merged q-window [qA(256) | qB(256)] per head; at/sps cols are
        # [h0: qA qB | h1: qA qB]. kci >= NKC0 touches only the qB halves.
        def run_attention(wv_t):
          with tc.tile_pool(name="msk", bufs=1) as mpool, \
             tc.tile_pool(name="atile", bufs=2) as apool, \
             tc.tile_pool(name="rec", bufs=1) as rpool, \
             tc.tile_pool(name="psD", bufs=1, space="PSUM") as psD, \
             tc.tile_pool(name="psCv", bufs=1, space="PSUM") as psCv, \
             tc.tile_pool(name="psS", bufs=2, space="PSUM") as psS:
            mk_t = [mpool.tile([P, 2 * QW], bf16, tag=f"m{u}", name=f"m{u}")
                    for u in range(NKC1)]
            for u in range(NKC1):
                nc.sync.dma_start(out=mk_t[u], in_=mk_d[u])
            # prefetch wp for phase E during attention
            for i in range(ND):
                nc.sync.dma_start(out=wp_t[i], in_=wp_d[P * i:P * i + P, :])

            # V-proj interleaves with attention (fills PE under exp-bound)
            for kc in range(NT):
                vps = psCv.tile([P, D], f32, tag="vps")
                for i in range(ND):
                    for nh in range(2):
                        nsl = slice(512 * nh, 512 * nh + 512)
                        nc.tensor.matmul(
                            vps[:, nsl],
                            hT[i][:, P * kc:P * kc + P],
                            wv_t[i][:, nsl],
                            start=(i == 0), stop=(i == ND - 1))
                nc.vector.memset(V65[kc][:, :, HS:HS + 1], 1.0)
                nc.vector.tensor_copy(
                    V65[kc][:, :, 0:HS],
                    vps.rearrange("p (h e) -> p h e", e=HS))

            W2Q = 2 * QW
            for pr in range(NPAIR):
                av = [psD.tile([HS + 1, W2Q], f32, tag=f"av{h}",
                               name=f"av{h}") for h in range(2)]
                for kci in range(NKC1):
                    sps = psS.tile([P, 2 * W2Q], f32, tag="sps", name="sps")
                    at = apool.tile([P, 2 * W2Q], bf16, tag="a", name="a")
                    if kci < NKC0:
                        # full q window for both heads
                        for h in range(2):
                            hb = slice(64 * h, 64 * h + 64)
                            nc.tensor.matmul(
                                sps[:, W2Q * h:W2Q * h + W2Q],
                                KT[pr][hb, P * kci:P * kci + P],
                                QT[pr][hb, :])
                        nc.scalar.activation(at, sps, Exp)
                        at_h = at.rearrange("p (h q) -> p h q", q=W2Q)
                        for h in range(2):
                            nc.vector.tensor_mul(at_h[:, h, :],
                                                 at_h[:, h, :], mk_t[kci])
                        for h in range(2):
                            nc.tensor.matmul(
                                av[h], V65[kci][:, 2 * pr + h, :],
                                at[:, W2Q * h:W2Q * h + W2Q],
                                start=(kci == 0), stop=(kci == NKC1 - 1))
                    else:
                        # only the qB half per head
                        for h in range(2):
                            hb = slice(64 * h, 64 * h + 64)
                            nc.tensor.matmul(
                                sps[:, W2Q * h + QW:W2Q * h + W2Q],
                                KT[pr][hb, P * kci:P * kci + P],
                                QT[pr][hb, QW:W2Q])
                        sps_b = sps.rearrange("p (h q) -> p h q", q=W2Q)
                        at_b = at.rearrange("p (h q) -> p h q", q=W2Q)
                        nc.scalar.activation(at_b[:, :, QW:W2Q],
                                             sps_b[:, :, QW:W2Q], Exp)
                        for h in range(2):
                            nc.vector.tensor_mul(at_b[:, h, QW:W2Q],
                                                 at_b[:, h, QW:W2Q],
                                                 mk_t[kci][:, QW:W2Q])
                        for h in range(2):
                            nc.tensor.matmul(
                                av[h][:, QW:W2Q],
                                V65[kci][:, 2 * pr + h, :],
                                at[:, W2Q * h + QW:W2Q * h + W2Q],
                                start=False, stop=(kci == NKC1 - 1))
                # normalize: one fast reciprocal + gpsimd broadcast per pair
                sums = rpool.tile([1, 2 * W2Q], f32, tag="sums")
                nc.vector.tensor_copy(sums[:, 0:W2Q], av[0][HS:HS + 1, :])
                nc.vector.tensor_copy(sums[:, W2Q:2 * W2Q],
                                      av[1][HS:HS + 1, :])
                rec = rpool.tile([1, 2 * W2Q], f32, tag="rec")
                nc.vector.reciprocal_approx_fast(rec, sums)
                rb_sb = rpool.tile([64, 2 * W2Q], f32, tag="rb_sb")
                nc.gpsimd.partition_broadcast(rb_sb, rec)
                for h in range(2):
                    hb = slice(64 * h, 64 * h + 64)
                    nc.vector.tensor_mul(attn[pr][hb, :], av[h][0:HS, :],
                                         rb_sb[:, W2Q * h:W2Q * h + W2Q])
                    nc.vector.tensor_scalar_add(
                        attn[pr][hb, :], attn[pr][hb, :],
                        bias["bv"][64 * h:64 * h + 64, pr:pr + 1])


        # ====== Phase B'/C interleaved: K-proj overlaps own-token LN ======
        with tc.tile_pool(name="wgt", bufs=1) as wpool:
          with tc.tile_pool(name="psC", bufs=3, space="PSUM") as psC, \
               tc.tile_pool(name="hq", bufs=1) as hq_pool:
            hq = [hq_pool.tile([P, OWN], bf16, tag=f"hq{i}", name=f"hq{i}")
                  for i in range(ND)]
            wk_t = [wpool.tile([P, D], bf16, tag=f"w{i}", name=f"wk{i}")
                    for i in range(ND)]
            for i in range(ND):
                nc.sync.dma_start(out=wk_t[i],
                                  in_=wk_d[P * i:P * i + P, :])

            with tc.tile_pool(name="sqB", bufs=4) as sqB, \
                 tc.tile_pool(name="htmB", bufs=3) as htB, \
                 tc.tile_pool(name="bcB", bufs=1) as bcB, \
                 tc.tile_pool(name="psA2", bufs=1, space="PSUM") as psA2:
                sx2 = psA2.tile([1, OWN], f32, tag="sx2")
                sq2 = psA2.tile([1, OWN], f32, tag="sq2")
                for i in range(ND):
                    sqt = sqB.tile([P, OWN], bf16, tag="sqt2")
                    nc.scalar.square(sqt, xo_t[i])
                    nc.tensor.matmul(sx2, ones_cf, xo_t[i],
                                     start=(i == 0), stop=(i == ND - 1))
                    nc.tensor.matmul(sq2, ones_bf, sqt,
                                     start=(i == 0), stop=(i == ND - 1))
                r_bf, s_bf = ln_rows(OWN, sx2, sq2)
                rb2 = bcB.tile([P, OWN], bf16, tag="rb2")
                nc.gpsimd.partition_broadcast(rb2, r_bf)
                sb2 = bcB.tile([P, OWN], bf16, tag="sb2")
                nc.gpsimd.partition_broadcast(sb2, s_bf)
                for i in range(ND):
                    xob = sqB.tile([P, OWN], bf16, tag="sqt2", name="xob")
                    nc.vector.tensor_copy(xob, xo_t[i])
                    tmp = htB.tile([P, OWN], bf16, tag="htmp2")
                    nc.vector.tensor_mul(tmp, xob, rb2)
                    nc.vector.tensor_add(hq[i], tmp, sb2)

                # K_T channel-major, all tokens (independent of hq chain)
                for pr in range(NPAIR):
                    for s in range(S // 512):
                        sl = slice(512 * s, 512 * s + 512)
                        kps = psC.tile([P, 512], f32, tag="kps")
                        for i in range(ND):
                            nc.tensor.matmul(
                                kps, wk_t[i][:, P * pr:P * pr + P],
                                hT[i][:, sl],
                                start=(i == 0), stop=(i == ND - 1))
                        nc.vector.tensor_scalar_add(
                            KT[pr][:, sl], kps, bias["bk"][:, pr:pr + 1])

            # Q_T from own tokens
            wq_t = [wpool.tile([P, D], bf16, tag=f"w{i}", name=f"wq{i}")
                    for i in range(ND)]
            for i in range(ND):
                nc.sync.dma_start(out=wq_t[i],
                                  in_=wq_d[P * i:P * i + P, :])
            for pr in range(NPAIR):
                qps = psC.tile([P, OWN], f32, tag="kps")
                for i in range(ND):
                    nc.tensor.matmul(qps,
                                     wq_t[i][:, P * pr:P * pr + P],
                                     hq[i],
                                     start=(i == 0), stop=(i == ND - 1))
                nc.vector.tensor_scalar_add(QT[pr], qps,
                                            bias["bq"][:, pr:pr + 1])
          # wv loaded here (tiles live into phase D for interleaved V-proj)
          wv_t = [wpool.tile([P, D], bf16, tag=f"w{i}", name=f"wv{i}")
                  for i in range(ND)]
          for i in range(ND):
              nc.sync.dma_start(out=wv_t[i],
                                in_=wv_d[P * i:P * i + P, :])

          # ---- phase D nested inside (wgt pool stays open for wv_t) ----
          run_attention(wv_t)

        pool_close(hT_cm)

        pool_close(qT_cm, v_cm, kT_cm)

        # right-stack pools for the token-parallel tail
        x2_cm, x2_pool = pool_open(name="x2", bufs=1, side="right")
        x2f = [x2_pool.tile([P, OWN], f32, tag=f"x2f{i}", name=f"x2f{i}")
               for i in range(ND)]
        x2b = [x2_pool.tile([P, OWN], bf16, tag=f"x2b{i}", name=f"x2b{i}")
               for i in range(ND)]

        # =========== Phase E: proj + residual ===========
        with tc.tile_pool(name="psE", bufs=2, space="PSUM") as psE:
            for mc in range(ND):
                ops = psE.tile([P, OWN], f32, tag="ops")
                for i in range(NPAIR):
                    nc.tensor.matmul(ops, wp_t[i][:, P * mc:P * mc + P],
                                     attn[i],
                                     start=(i == 0), stop=(i == NPAIR - 1))
                nc.scalar.activation(x2f[mc], ops, Identity,
                                     bias=bias["bp"][:, mc:mc + 1])
                nc.vector.tensor_add(x2f[mc], x2f[mc], xo_t[mc])
                nc.vector.tensor_copy(x2b[mc], x2f[mc])

        pool_close(at_cm)

        h2_cm, h2_pool = pool_open(name="h2", bufs=1, side="right")
        h2 = [h2_pool.tile([P, OWN], bf16, tag=f"h2{i}", name=f"h2{i}")
              for i in range(ND)]

        # =========== Phase F: LN2 ===========
        with tc.tile_pool(name="sqF", bufs=3, side="right") as sqF, \
             tc.tile_pool(name="htmF", bufs=3, side="right") as htF, \
             tc.tile_pool(name="bcF", bufs=1, side="right") as bcF, \
             tc.tile_pool(name="psF", bufs=1, space="PSUM") as psF:
            sx2 = psF.tile([1, OWN], f32, tag="sxF")
            sq2 = psF.tile([1, OWN], f32, tag="sqF")
            for i in range(ND):
                sqt = sqF.tile([P, OWN], bf16, tag="sqtF")
                nc.scalar.square(sqt, x2b[i])
                nc.tensor.matmul(sx2, ones_bf, x2b[i],
                                 start=(i == 0), stop=(i == ND - 1))
                nc.tensor.matmul(sq2, ones_bf, sqt,
                                 start=(i == 0), stop=(i == ND - 1))
            r_bf, s_bf = ln_rows(OWN, sx2, sq2)
            rbF = bcF.tile([P, OWN], bf16, tag="rbF")
            nc.gpsimd.partition_broadcast(rbF, r_bf)
            sbF = bcF.tile([P, OWN], bf16, tag="sbF")
            nc.gpsimd.partition_broadcast(sbF, s_bf)
            for i in range(ND):
                tmp = htF.tile([P, OWN], bf16, tag="htmpF")
                nc.vector.tensor_mul(tmp, x2b[i], rbF)
                nc.vector.tensor_add(h2[i], tmp, sbF)

        # =========== Phase G: FFN ===========
        with tc.tile_pool(name="gbuf", bufs=1, side="right") as g_pool, \
             tc.tile_pool(name="w1c", bufs=3, side="right") as w1pool, \
             tc.tile_pool(name="w2c", bufs=2, side="right") as w2pool, \
             tc.tile_pool(name="outp", bufs=4, side="right") as opool, \
             tc.tile_pool(name="psG", bufs=2, space="PSUM") as psG:
            g = [g_pool.tile([P, OWN], bf16, tag=f"g{m}", name=f"g{m}")
                 for m in range(NFF)]
            w1r = w1_d.rearrange("(ko ki) f -> ki ko f", ki=P)
            for mc in range(NFF):
                w1c = w1pool.tile([P, ND, P], bf16, tag="w1c")
                nc.sync.dma_start(out=w1c, in_=w1r[:, :, P * mc:P * mc + P])
                ups = psG.tile([P, OWN], f32, tag="ups")
                for i in range(ND):
                    nc.tensor.matmul(ups, w1c[:, i, :], h2[i],
                                     start=(i == 0), stop=(i == ND - 1))
                nc.scalar.activation(g[mc], ups, Gelu,
                                     bias=bias["b1"][:, mc:mc + 1])
            w2r = w2_d.rearrange("(ko ki) f -> ki ko f", ki=P)
            for oc in range(ND):
                w2c = w2pool.tile([P, NFF, P], bf16, tag="w2c")
                nc.sync.dma_start(out=w2c, in_=w2r[:, :, P * oc:P * oc + P])
                wps = psG.tile([P, OWN], f32, tag="ups")
                for k in range(NFF):
                    nc.tensor.matmul(wps, w2c[:, k, :], g[k],
                                     start=(k == 0), stop=(k == NFF - 1))
                of = opool.tile([P, OWN], f32, tag="of")
                nc.scalar.activation(of, wps, Identity,
                                     bias=bias["b2"][:, oc:oc + 1])
                nc.vector.tensor_add(of, of, x2f[oc])
                nc.sync.dma_start(out=out_d[P * oc:P * oc + P, :], in_=of)

        pool_close(h2_cm, x2_cm, wgtE_cm, xo_cm)

    return nc


def host_prep(inputs):
    """Build per-core input maps + gather metadata. Pure numpy."""
    x = np.asarray(inputs["x"], np.float32)
    ln1_w = np.asarray(inputs["ln1_w"], np.float32)
    ln1_b = np.asarray(inputs["ln1_b"], np.float32)
    ln2_w = np.asarray(inputs["ln2_w"], np.float32)
    ln2_b = np.asarray(inputs["ln2_b"], np.float32)

    def cat_heads(w):
        return np.ascontiguousarray(
            np.transpose(np.asarray(w, np.float32), (1, 0, 2)).reshape(D, D))

    wq_c, wk_c, wv_c = (cat_heads(inputs[k]) for k in ("Wq", "Wk", "Wv"))
    bq_f = np.asarray(inputs["bq"], np.float32).reshape(-1)
    bk_f = np.asarray(inputs["bk"], np.float32).reshape(-1)
    bv_f = np.asarray(inputs["bv"], np.float32).reshape(-1)
    Wp = np.asarray(inputs["Wp"], np.float32)
    bp = np.asarray(inputs["bp"], np.float32)
    W1 = np.asarray(inputs["W1"], np.float32)
    b1 = np.asarray(inputs["b1"], np.float32)
    W2 = np.asarray(inputs["W2"], np.float32)
    b2 = np.asarray(inputs["b2"], np.float32)

    sc = 1.0 / np.sqrt(HS)
    wq_eff = ((ln1_w[:, None] * wq_c) * sc).astype(BF16)
    bq_eff = ((ln1_b @ wq_c + bq_f) * sc).astype(np.float32)
    wk_eff = (ln1_w[:, None] * wk_c).astype(BF16)
    bk_eff = (ln1_b @ wk_c + bk_f).astype(np.float32)
    wv_eff = (ln1_w[:, None] * wv_c).astype(BF16)
    bv_eff = (ln1_b @ wv_c + bv_f).astype(np.float32)
    wp_eff = Wp.astype(BF16)
    w1_eff = (ln2_w[:, None] * W1).astype(BF16)
    b1_eff = (ln2_b @ W1 + b1).astype(np.float32)
    w2_eff = W2.astype(BF16)

    def chunked(v, n):
        return np.ascontiguousarray(v.reshape(n, P).T).astype(np.float32)

    shared = {
        "wq": wq_eff, "wk": wk_eff, "wv": wv_eff, "wp": wp_eff,
        "w1": w1_eff, "w2": w2_eff,
        "bq": chunked(bq_eff, ND), "bk": chunked(bk_eff, ND),
        "bv": chunked(bv_eff, ND), "bp": chunked(bp, ND),
        "b1": chunked(b1_eff, NFF), "b2": chunked(b2, ND),
    }

    in_maps, gathers = [], []
    for c in range(N_CORES):
        b, j = c // 4, c % 4
        qA, qB = QW * j, QW * (7 - j)
        xT = np.ascontiguousarray(x[b].T).astype(BF16)
        xo = np.ascontiguousarray(
            np.concatenate([x[b, qA:qA + QW].T, x[b, qB:qB + QW].T],
                           axis=1)).astype(np.float32)
        # masks[kc][:, h*512 + 0:256] = causal(kc, qA); [256:512] = (kc, qB)
        ks = np.arange(P)[:, None]
        qs = np.arange(QW)[None, :]
        masks = np.zeros((NKC1, P, 2 * QW), np.float32)
        for kc in range(NKC1):
            masks[kc, :, 0:QW] = (P * kc + ks) <= (qA + qs)
            masks[kc, :, QW:2 * QW] = (P * kc + ks) <= (qB + qs)
        m = dict(shared)
        m["xT"] = xT
        m["xo"] = xo
        m["masks"] = masks.astype(BF16)
        in_maps.append(m)
        gathers.append((b, qA, qB))
    return in_maps, gathers


def make_nc():
    from concourse import bacc

    nc = bacc.Bacc("TRN2")
    build(nc)
    nc.compile()
    return nc


def kernel(**inputs):
    from concourse.bass_utils import run_bass_kernel_spmd

    nc = make_nc()
    in_maps, gathers = host_prep(inputs)
    res = run_bass_kernel_spmd(nc, in_maps, list(range(N_CORES)))
    out = np.zeros((B, S, D), np.float32)
    for c, (b, qA, qB) in enumerate(gathers):
        oT = res.results[c]["outT"]
        out[b, qA:qA + QW] = oT[:, 0:QW].T
        out[b, qB:qB + QW] = oT[:, QW:2 * QW].T
    return out



# revision 20
# speedup vs baseline: 1.1660x; 1.1660x over previous
"""Trainium2 Bass kernel for a dense transformer block.

Layout strategy: channel-major activations ([d, tokens]) so every linear
layer is a natural PE matmul (contraction dim on partitions, weights in
natural [d_in, d_out] layout as lhsT). Softmax is computed transposed
(S^T = [key, q]) without max-subtraction (scores bounded), with row-sums
obtained from a ones-column appended to V during the A@V matmul.

Sharding over 8 cores, no collectives: core c -> batch b=c//4, query
chunks {j, 7-j} (j=c%4, 256 tokens each). LN1/K/V computed redundantly
for the full batch on each core; causality via per-core mask inputs so
the compiled program is identical on all cores (single-NEFF SPMD).

Scheduling notes (from perfetto traces):
- K-projection is interleaved into the LN1 per-chunk loop and V-proj
  for the second half of the keys into the attention pair loop, so the
  PE never idles long enough for the HAM clock gate to re-throttle it
  to 1.2 GHz.
- Attention is split D1 (kci<8, full q windows) / D2 (kci>=8, qB only);
  each pair's normalize tail is software-pipelined into the next pair.
- The qB halves need no masking in D1 (all keys < 1024 are causally
  valid for tokens at positions >= 1024).
- All engines throttle ~20% chip-wide (P0 power state) if every engine
  is kept dense simultaneously -- reducing total work beats packing
  for density.
"""

import numpy as np
import ml_dtypes

# Problem constants (hardcoded per task contract)
B, S, D, H, HS, FF = 2, 2048, 1024, 16, 64, 4096
P = 128
ND = D // P          # 8 d-chunks
NT = S // P          # 16 key chunks
NPAIR = H // 2       # 8 head pairs
QW = 256             # query chunk width
OWN = 2 * QW         # 512 owned query tokens per core
NKC0, NKC1 = 8, 16   # uniform key-chunk counts for q-chunk 0 / 1
NFF = FF // P        # 32
NO = OWN // 512      # 1 (owned tokens fit one 512 slice)
EPS = 1e-5
N_CORES = 8

BF16 = ml_dtypes.bfloat16


def build(nc):
    """Build the single-core SPMD program (identical for all cores)."""
    import concourse.mybir as mybir
    from concourse.tile import TileContext
    from contextlib import ExitStack

    dt = mybir.dt
    f32, bf16 = dt.float32, dt.bfloat16
    Exp = mybir.ActivationFunctionType.Exp
    Gelu = mybir.ActivationFunctionType.Gelu
    Sqrt = mybir.ActivationFunctionType.Sqrt
    Identity = mybir.ActivationFunctionType.Identity

    # ---- I/O ----
    xT_d = nc.dram_tensor("xT", [D, S], bf16, kind="ExternalInput")
    xo_d = nc.dram_tensor("xo", [D, OWN], f32, kind="ExternalInput")
    wq_d = nc.dram_tensor("wq", [D, D], bf16, kind="ExternalInput")
    wk_d = nc.dram_tensor("wk", [D, D], bf16, kind="ExternalInput")
    wv_d = nc.dram_tensor("wv", [D, D], bf16, kind="ExternalInput")
    wp_d = nc.dram_tensor("wp", [D, D], bf16, kind="ExternalInput")
    w1_d = nc.dram_tensor("w1", [D, FF], bf16, kind="ExternalInput")
    w2_d = nc.dram_tensor("w2", [FF, D], bf16, kind="ExternalInput")
    bq_d = nc.dram_tensor("bq", [P, ND], f32, kind="ExternalInput")
    bk_d = nc.dram_tensor("bk", [P, ND], f32, kind="ExternalInput")
    bv_d = nc.dram_tensor("bv", [P, ND], f32, kind="ExternalInput")
    bp_d = nc.dram_tensor("bp", [P, ND], f32, kind="ExternalInput")
    b1_d = nc.dram_tensor("b1", [P, NFF], f32, kind="ExternalInput")
    b2_d = nc.dram_tensor("b2", [P, ND], f32, kind="ExternalInput")
    mk_d = nc.dram_tensor("masks", [NKC1, P, 2 * QW], bf16,
                          kind="ExternalInput")
    out_d = nc.dram_tensor("outT", [D, OWN], f32, kind="ExternalOutput")

    with TileContext(nc) as tc, ExitStack() as top:
        const = top.enter_context(tc.tile_pool(name="const", bufs=1))
        rowp = top.enter_context(tc.tile_pool(name="rows", bufs=1))

        ones_bf = const.tile([P, 1], bf16)
        nc.vector.memset(ones_bf, 1.0)
        ones_f = const.tile([1, P], f32)
        nc.vector.memset(ones_f, 1.0)
        eps_t = const.tile([1, 1], f32)
        nc.vector.memset(eps_t, EPS)
        ones_cf = const.tile([P, 1], f32)
        nc.vector.memset(ones_cf, 1.0)

        bias = {}
        for name, dram, w in (("bq", bq_d, ND), ("bk", bk_d, ND),
                              ("bv", bv_d, ND), ("bp", bp_d, ND),
                              ("b1", b1_d, NFF), ("b2", b2_d, ND)):
            t = const.tile([P, w], f32, tag=f"bias_{name}", name=f"bias_{name}")
            nc.sync.dma_start(out=t, in_=dram[:, :])
            bias[name] = t

        def pool_open(**kw):
            cm = tc.tile_pool(**kw)
            return cm, cm.__enter__()

        def pool_close(*cms):
            for cm in cms:
                cm.__exit__(None, None, None)

        def ln_rows(n, sx_ps, sq_ps):
            """row stats [1, n] from Sigma-x / Sigma-x2 PSUM -> (r_row, s_row).
            Tags shared across phases (sequential use)."""
            mean = rowp.tile([1, n], f32, tag="mean", name="mean")
            nc.scalar.mul(mean, sx_ps, 1.0 / D)
            negm = rowp.tile([1, n], f32, tag="negm", name="negm")
            nc.scalar.mul(negm, sx_ps, -1.0 / D)
            var = rowp.tile([1, n], f32, tag="var", name="var")
            nc.scalar.mul(var, sq_ps, 1.0 / D)
            msq = rowp.tile([1, n], f32, tag="msq", name="msq")
            nc.vector.tensor_mul(msq, mean, mean)
            nc.vector.tensor_sub(var, var, msq)
            std = rowp.tile([1, n], f32, tag="std", name="std")
            nc.scalar.activation(std, var, Sqrt, bias=eps_t)
            r_row = rowp.tile([1, n], f32, tag="r_row", name="r_row")
            nc.vector.reciprocal_approx_fast(r_row, std)
            s_row = rowp.tile([1, n], f32, tag="s_row", name="s_row")
            nc.vector.tensor_mul(s_row, negm, r_row)
            r_bf = rowp.tile([1, n], bf16, tag="r_bf", name="r_bf")
            nc.vector.tensor_copy(r_bf, r_row)
            s_bf = rowp.tile([1, n], bf16, tag="s_bf", name="s_bf")
            nc.vector.tensor_copy(s_bf, s_row)
            return r_bf, s_bf

        # ---------- long-lived left-stack pools ----------
        xo_cm, xo_pool = pool_open(name="xo", bufs=1, side="right")
        at_cm, at_pool = pool_open(name="attn", bufs=1)
        xo_t = [xo_pool.tile([P, OWN], f32, tag=f"xo{i}", name=f"xo{i}")
                for i in range(ND)]
        attn = [at_pool.tile([P, OWN], bf16, tag=f"at{p}", name=f"at{p}")
                for p in range(NPAIR)]
        # wgtE pool: holds wv during phases C..D1, then wp (phase E)
        wgtE_cm, wpoolE = pool_open(name="wgtE", bufs=1, side="right")

        # hT on the right stack: lives phases A..C only
        hT_cm, hT_pool = pool_open(name="hT", bufs=1, side="right")
        hT = [hT_pool.tile([P, S], bf16, tag=f"h{i}", name=f"h{i}")
              for i in range(ND)]

        # ---------- K/V/Q pools (left), live through phase D ----------
        kT_cm, kT_pool = pool_open(name="kT", bufs=1)
        v_cm, v_pool = pool_open(name="v65", bufs=1)
        qT_cm, qT_pool = pool_open(name="qT", bufs=1)
        KT = [kT_pool.tile([P, S], bf16, tag=f"k{p}", name=f"k{p}")
              for p in range(NPAIR)]
        V65 = [v_pool.tile([P, H, HS + 1], bf16, tag=f"v{k}", name=f"v{k}")
               for k in range(NT)]
        QT = [qT_pool.tile([P, OWN], bf16, tag=f"q{p}", name=f"q{p}")
              for p in range(NPAIR)]

        # =========== Phase D: attention (fn; called inside C scope) =====
        # merged q-window [qA(256) | qB(256)] per head; at/sps cols are
        # [h0: qA qB | h1: qA qB]. Split into D1 (kci<8, full windows) and
        # D2 (kci>=8, qB halves + normalize). V-proj for kc 8..15 is
        # interleaved into D1's pair loop so the PE never idles (keeps the
        # HAM clock warm); D1 partial sums flush to SBUF (bf16) to free
        # PSUM banks for the next pair.
        def run_attention(wv_t):
          W2Q = 2 * QW
          with tc.tile_pool(name="msk", bufs=1) as mpool, \
             tc.tile_pool(name="atile", bufs=2) as apool, \
             tc.tile_pool(name="av1", bufs=1) as av1pool, \
             tc.tile_pool(name="rec", bufs=1) as rpool, \
             tc.tile_pool(name="psD", bufs=1, space="PSUM") as psD, \
             tc.tile_pool(name="psCv", bufs=1, space="PSUM") as psCv, \
             tc.tile_pool(name="psS", bufs=2, space="PSUM") as psS:
            # D1 (kci<8): only the qA half ever needs masking (qB tokens
            # sit at positions >= 1024 so keys 0..1023 are all valid).
            # D2 (kci>=8): only the qB half is processed.
            mk_t = [mpool.tile([P, QW], bf16, tag=f"m{u}", name=f"m{u}")
                    for u in range(NKC1)]
            for u in range(NKC0):
                nc.sync.dma_start(out=mk_t[u], in_=mk_d[u][:, 0:QW])
            for u in range(NKC0, NKC1):
                nc.sync.dma_start(out=mk_t[u], in_=mk_d[u][:, QW:2 * QW])

            # qB-half partial sums from D1 (qA needs no D2 contribution and
            # is normalized at the end of D1)
            avs1 = [[av1pool.tile([HS + 1, QW], bf16, tag=f"avs{pr}_{h}",
                                  name=f"avs{pr}_{h}") for h in range(2)]
                    for pr in range(NPAIR)]

            def vproj_mm(kc, i, vps):
                for nh in range(2):
                    nsl = slice(512 * nh, 512 * nh + 512)
                    nc.tensor.matmul(
                        vps[:, nsl],
                        hT[i][:, P * kc:P * kc + P],
                        wv_t[i][:, nsl],
                        start=(i == 0), stop=(i == ND - 1))

            def vproj_out(kc, vps):
                nc.vector.memset(V65[kc][:, :, HS:HS + 1], 1.0)
                nc.vector.tensor_copy(
                    V65[kc][:, :, 0:HS],
                    vps.rearrange("p (h e) -> p h e", e=HS))

            # V-proj for the first half of the keys (kc 0..7) up front
            for kc in range(NKC0):
                vps = psCv.tile([P, D], f32, tag="vps")
                for i in range(ND):
                    vproj_mm(kc, i, vps)
                vproj_out(kc, vps)

            # ---- D1: kci 0..7, full q windows; V-proj kc 8..15 woven in
            for pr in range(NPAIR):
                kc2 = NKC0 + pr
                vps = psCv.tile([P, D], f32, tag="vps")
                av = [psD.tile([HS + 1, W2Q], f32, tag=f"av{h}",
                               name=f"av{h}") for h in range(2)]
                for kci in range(NKC0):
                    vproj_mm(kc2, kci, vps)
                    sps = psS.tile([P, 2 * W2Q], f32, tag="sps", name="sps")
                    at = apool.tile([P, 2 * W2Q], bf16, tag="a", name="a")
                    for h in range(2):
                        hb = slice(64 * h, 64 * h + 64)
                        nc.tensor.matmul(
                            sps[:, W2Q * h:W2Q * h + W2Q],
                            KT[pr][hb, P * kci:P * kci + P],
                            QT[pr][hb, :])
                    nc.scalar.activation(at, sps, Exp)
                    at_h = at.rearrange("p (h q) -> p h q", q=W2Q)
                    for h in range(2):
                        nc.vector.tensor_mul(at_h[:, h, :],
                                             at_h[:, h, :], mk_t[kci])
                    for h in range(2):
                        nc.tensor.matmul(
                            av[h], V65[kci][:, 2 * pr + h, :],
                            at[:, W2Q * h:W2Q * h + W2Q],
                            start=(kci == 0), stop=(kci == NKC0 - 1))
                vproj_out(kc2, vps)
                # flush qB partials to SBUF; normalize the finished qA half
                for h in range(2):
                    nc.vector.tensor_copy(avs1[pr][h], av[h][:, QW:W2Q])
                sums = rpool.tile([1, 2 * QW], f32, tag="sums")
                for h in range(2):
                    nc.vector.tensor_copy(sums[:, QW * h:QW * h + QW],
                                          av[h][HS:HS + 1, 0:QW])
                rec = rpool.tile([1, 2 * QW], f32, tag="rec")
                nc.vector.reciprocal_approx_fast(rec, sums)
                rb_sb = rpool.tile([64, 2 * QW], f32, tag="rb_sb")
                nc.gpsimd.partition_broadcast(rb_sb, rec)
                for h in range(2):
                    hb = slice(64 * h, 64 * h + 64)
                    nc.vector.tensor_mul(attn[pr][hb, 0:QW],
                                         av[h][0:HS, 0:QW],
                                         rb_sb[:, QW * h:QW * h + QW])
                    nc.vector.tensor_scalar_add(
                        attn[pr][hb, 0:QW], attn[pr][hb, 0:QW],
                        bias["bv"][64 * h:64 * h + 64, pr:pr + 1])

            # ---- D2: kci 8..15, qB halves only, then normalize
            for pr in range(NPAIR):
                av2 = [psD.tile([HS + 1, W2Q], f32, tag=f"av{h}",
                                name=f"av2_{h}") for h in range(2)]
                for kci in range(NKC0, NKC1):
                    sps = psS.tile([P, 2 * W2Q], f32, tag="sps",
                                   name="sps2")
                    at = apool.tile([P, 2 * W2Q], bf16, tag="a", name="a2")
                    for h in range(2):
                        hb = slice(64 * h, 64 * h + 64)
                        nc.tensor.matmul(
                            sps[:, W2Q * h:W2Q * h + QW],
                            KT[pr][hb, P * kci:P * kci + P],
                            QT[pr][hb, QW:W2Q])
                    at_h = at.rearrange("p (h q) -> p h q", q=W2Q)
                    sps_h = sps.rearrange("p (h q) -> p h q", q=W2Q)
                    nc.scalar.activation(at_h[:, :, 0:QW],
                                         sps_h[:, :, 0:QW], Exp)
                    for h in range(2):
                        nc.vector.tensor_mul(at_h[:, h, 0:QW],
                                             at_h[:, h, 0:QW], mk_t[kci])
                    for h in range(2):
                        nc.tensor.matmul(
                            av2[h][:, 0:QW], V65[kci][:, 2 * pr + h, :],
                            at[:, W2Q * h:W2Q * h + QW],
                            start=(kci == NKC0), stop=(kci == NKC1 - 1))
                # combine with D1's qB partials, then normalize
                avB = [rpool.tile([HS + 1, QW], bf16, tag=f"avB{h}",
                                  name=f"avB{h}") for h in range(2)]
                for h in range(2):
                    nc.vector.tensor_add(avB[h], avs1[pr][h],
                                         av2[h][:, 0:QW])
                sums = rpool.tile([1, 2 * QW], f32, tag="sums")
                for h in range(2):
                    nc.vector.tensor_copy(sums[:, QW * h:QW * h + QW],
                                          avB[h][HS:HS + 1, :])
                rec = rpool.tile([1, 2 * QW], f32, tag="rec")
                nc.vector.reciprocal_approx_fast(rec, sums)
                rb_sb = rpool.tile([64, 2 * QW], f32, tag="rb_sb")
                nc.gpsimd.partition_broadcast(rb_sb, rec)
                for h in range(2):
                    hb = slice(64 * h, 64 * h + 64)
                    nc.vector.tensor_mul(attn[pr][hb, QW:W2Q],
                                         avB[h][0:HS, :],
                                         rb_sb[:, QW * h:QW * h + QW])
                    nc.vector.tensor_scalar_add(
                        attn[pr][hb, QW:W2Q], attn[pr][hb, QW:W2Q],
                        bias["bv"][64 * h:64 * h + 64, pr:pr + 1])


        # ====== Phase A+C1: LN1 over full batch, K-proj interleaved =====
        # x streams per 512-token chunk into recycling slots; each chunk's
        # K-projection fills the PE while the next chunk's LN row chain
        # runs on Vector/Scalar. A burst of junk matmuls at t=0 lifts the
        # PE clock gate (HAM) out of its cold 1.2 GHz state before the
        # real work arrives.
        with tc.tile_pool(name="wgt", bufs=1) as wpool:
          wk_t = [wpool.tile([P, D], bf16, tag=f"w{i}", name=f"wk{i}")
                  for i in range(ND)]
          with tc.tile_pool(name="xin", bufs=2, side="right") as x_pool, \
               tc.tile_pool(name="sq", bufs=3, side="right") as sq_pool, \
               tc.tile_pool(name="htm", bufs=2, side="right") as ht_pool, \
               tc.tile_pool(name="bcA", bufs=2, side="right") as bc_pool, \
               tc.tile_pool(name="psA", bufs=2, space="PSUM") as psA, \
               tc.tile_pool(name="psC", bufs=3, space="PSUM") as psC:
            xts = {}
            for s in range(S // 512):
                for i in range(ND):
                    t = x_pool.tile([P, 512], bf16, tag=f"x{i}",
                                    name=f"x{i}_{s}")
                    nc.sync.dma_start(
                        out=t,
                        in_=xT_d[P * i:P * i + P, 512 * s:512 * s + 512])
                    xts[(i, s)] = t
            for i in range(ND):
                nc.sync.dma_start(out=xo_t[i], in_=xo_d[P * i:P * i + P, :])
            for i in range(ND):
                nc.sync.dma_start(out=wk_t[i],
                                  in_=wk_d[P * i:P * i + P, :])

            def stats(s):
                sx_ps = psA.tile([1, 512], f32, tag="sx", name=f"sx{s}")
                sq_ps = psA.tile([1, 512], f32, tag="sq", name=f"sq{s}")
                for i in range(ND):
                    sqt = sq_pool.tile([P, 512], bf16, tag="sqt")
                    nc.vector.tensor_mul(sqt, xts[(i, s)], xts[(i, s)])
                    nc.tensor.matmul(sx_ps, ones_bf, xts[(i, s)],
                                     start=(i == 0), stop=(i == ND - 1))
                    nc.tensor.matmul(sq_ps, ones_bf, sqt,
                                     start=(i == 0), stop=(i == ND - 1))
                return sx_ps, sq_ps

            st = stats(0)
            for s in range(S // 512):
                sl = slice(512 * s, 512 * s + 512)
                r_bf, s_bf = ln_rows(512, *st)
                rb = bc_pool.tile([P, 512], bf16, tag="rb")
                nc.gpsimd.partition_broadcast(rb, r_bf)
                sb = bc_pool.tile([P, 512], bf16, tag="sb")
                nc.gpsimd.partition_broadcast(sb, s_bf)
                if s + 1 < S // 512:
                    st = stats(s + 1)
                for i in range(ND):
                    tmp = ht_pool.tile([P, 512], bf16, tag="htmp")
                    nc.vector.tensor_mul(tmp, xts[(i, s)], rb)
                    nc.vector.tensor_add(hT[i][:, sl], tmp, sb)
                # K-proj for this chunk (PE work under the next LN chain)
                for pr in range(NPAIR):
                    kps = psC.tile([P, 512], f32, tag="kps")
                    for i in range(ND):
                        nc.tensor.matmul(
                            kps, wk_t[i][:, P * pr:P * pr + P],
                            hT[i][:, sl],
                            start=(i == 0), stop=(i == ND - 1))
                    nc.vector.tensor_scalar_add(
                        KT[pr][:, sl], kps, bias["bk"][:, pr:pr + 1])

          # ====== Phase B'/C2: own-token LN + Q-proj ======
          with tc.tile_pool(name="psC2", bufs=3, space="PSUM") as psC2, \
               tc.tile_pool(name="hq", bufs=1) as hq_pool:
            hq = [hq_pool.tile([P, OWN], bf16, tag=f"hq{i}", name=f"hq{i}")
                  for i in range(ND)]
            wq_t = [wpool.tile([P, D], bf16, tag=f"w{i}", name=f"wq{i}")
                    for i in range(ND)]
            for i in range(ND):
                nc.sync.dma_start(out=wq_t[i],
                                  in_=wq_d[P * i:P * i + P, :])
            # wv streams into the wgtE slots (wp reloads them in D2)
            wv_t = [wpoolE.tile([P, D], bf16, tag=f"wp{i}", name=f"wv{i}")
                    for i in range(ND)]
            for i in range(ND):
                nc.sync.dma_start(out=wv_t[i],
                                  in_=wv_d[P * i:P * i + P, :])

            with tc.tile_pool(name="sqB", bufs=4) as sqB, \
                 tc.tile_pool(name="htmB", bufs=3) as htB, \
                 tc.tile_pool(name="bcB", bufs=1) as bcB, \
                 tc.tile_pool(name="psA2", bufs=1, space="PSUM") as psA2:
                sx2 = psA2.tile([1, OWN], f32, tag="sx2")
                sq2 = psA2.tile([1, OWN], f32, tag="sq2")
                for i in range(ND):
                    sqt = sqB.tile([P, OWN], bf16, tag="sqt2")
                    nc.vector.tensor_mul(sqt, xo_t[i], xo_t[i])
                    nc.tensor.matmul(sx2, ones_cf, xo_t[i],
                                     start=(i == 0), stop=(i == ND - 1))
                    nc.tensor.matmul(sq2, ones_bf, sqt,
                                     start=(i == 0), stop=(i == ND - 1))
                r_bf, s_bf = ln_rows(OWN, sx2, sq2)
                rb2 = bcB.tile([P, OWN], bf16, tag="rb2")
                nc.gpsimd.partition_broadcast(rb2, r_bf)
                sb2 = bcB.tile([P, OWN], bf16, tag="sb2")
                nc.gpsimd.partition_broadcast(sb2, s_bf)
                for i in range(ND):
                    xob = sqB.tile([P, OWN], bf16, tag="sqt2", name="xob")
                    nc.vector.tensor_copy(xob, xo_t[i])
                    tmp = htB.tile([P, OWN], bf16, tag="htmp2")
                    nc.vector.tensor_mul(tmp, xob, rb2)
                    nc.vector.tensor_add(hq[i], tmp, sb2)

            # Q_T from own tokens
            for pr in range(NPAIR):
                qps = psC2.tile([P, OWN], f32, tag="kps")
                for i in range(ND):
                    nc.tensor.matmul(qps,
                                     wq_t[i][:, P * pr:P * pr + P],
                                     hq[i],
                                     start=(i == 0), stop=(i == ND - 1))
                nc.vector.tensor_scalar_add(QT[pr], qps,
                                            bias["bq"][:, pr:pr + 1])

          # ---- phase D nested inside (wgtE pool holds wv_t) ----
          wp_t = run_attention(wv_t)

        pool_close(hT_cm)

        pool_close(qT_cm, v_cm, kT_cm)

        # right-stack pools for the token-parallel tail
        x2_cm, x2_pool = pool_open(name="x2", bufs=1, side="right")
        x2f = [x2_pool.tile([P, OWN], f32, tag=f"x2f{i}", name=f"x2f{i}")
               for i in range(ND)]
        x2b = [x2_pool.tile([P, OWN], bf16, tag=f"x2b{i}", name=f"x2b{i}")
               for i in range(ND)]

        # =========== Phase E: proj + residual ===========
        with tc.tile_pool(name="psE", bufs=2, space="PSUM") as psE:
            for mc in range(ND):
                ops = psE.tile([P, OWN], f32, tag="ops")
                for i in range(NPAIR):
                    nc.tensor.matmul(ops, wp_t[i][:, P * mc:P * mc + P],
                                     attn[i],
                                     start=(i == 0), stop=(i == NPAIR - 1))
                nc.scalar.activation(x2f[mc], ops, Identity,
                                     bias=bias["bp"][:, mc:mc + 1])
                nc.vector.tensor_add(x2f[mc], x2f[mc], xo_t[mc])
                nc.vector.tensor_copy(x2b[mc], x2f[mc])

        pool_close(at_cm)

        h2_cm, h2_pool = pool_open(name="h2", bufs=1, side="right")
        h2 = [h2_pool.tile([P, OWN], bf16, tag=f"h2{i}", name=f"h2{i}")
              for i in range(ND)]

        # =========== Phase F: LN2 ===========
        with tc.tile_pool(name="sqF", bufs=3, side="right") as sqF, \
             tc.tile_pool(name="htmF", bufs=3, side="right") as htF, \
             tc.tile_pool(name="bcF", bufs=1, side="right") as bcF, \
             tc.tile_pool(name="psF", bufs=1, space="PSUM") as psF:
            sx2 = psF.tile([1, OWN], f32, tag="sxF")
            sq2 = psF.tile([1, OWN], f32, tag="sqF")
            for i in range(ND):
                sqt = sqF.tile([P, OWN], bf16, tag="sqtF")
                nc.vector.tensor_mul(sqt, x2b[i], x2b[i])
                nc.tensor.matmul(sx2, ones_bf, x2b[i],
                                 start=(i == 0), stop=(i == ND - 1))
                nc.tensor.matmul(sq2, ones_bf, sqt,
                                 start=(i == 0), stop=(i == ND - 1))
            r_bf, s_bf = ln_rows(OWN, sx2, sq2)
            rbF = bcF.tile([P, OWN], bf16, tag="rbF")
            nc.gpsimd.partition_broadcast(rbF, r_bf)
            sbF = bcF.tile([P, OWN], bf16, tag="sbF")
            nc.gpsimd.partition_broadcast(sbF, s_bf)
            for i in range(ND):
                tmp = htF.tile([P, OWN], bf16, tag="htmpF")
                nc.vector.tensor_mul(tmp, x2b[i], rbF)
                nc.vector.tensor_add(h2[i], tmp, sbF)

        # =========== Phase G: FFN ===========
        with tc.tile_pool(name="gbuf", bufs=1, side="right") as g_pool, \
             tc.tile_pool(name="w1c", bufs=8, side="right") as w1pool, \
             tc.tile_pool(name="w2c", bufs=2, side="right") as w2pool, \
             tc.tile_pool(name="outp", bufs=4, side="right") as opool, \
             tc.tile_pool(name="psG", bufs=2, space="PSUM") as psG:
            g = [g_pool.tile([P, OWN], bf16, tag=f"g{m}", name=f"g{m}")
                 for m in range(NFF)]
            w1r = w1_d.rearrange("(ko ki) f -> ki ko f", ki=P)
            w1cs = []
            for mc in range(NFF):
                w1c = w1pool.tile([P, ND, P], bf16, tag="w1c",
                                  name=f"w1c{mc}")
                nc.sync.dma_start(out=w1c, in_=w1r[:, :, P * mc:P * mc + P])
                w1cs.append(w1c)
            for mc in range(NFF):
                ups = psG.tile([P, OWN], f32, tag="ups")
                for i in range(ND):
                    nc.tensor.matmul(ups, w1cs[mc][:, i, :], h2[i],
                                     start=(i == 0), stop=(i == ND - 1))
                nc.scalar.activation(g[mc], ups, Gelu,
                                     bias=bias["b1"][:, mc:mc + 1])
            w2r = w2_d.rearrange("(ko ki) f -> ki ko f", ki=P)
            for oc in range(ND):
                w2c = w2pool.tile([P, NFF, P], bf16, tag="w2c")
                nc.sync.dma_start(out=w2c, in_=w2r[:, :, P * oc:P * oc + P])
                wps = psG.tile([P, OWN], f32, tag="ups")
                for k in range(NFF):
                    nc.tensor.matmul(wps, w2c[:, k, :], g[k],
                                     start=(k == 0), stop=(k == NFF - 1))
                of = opool.tile([P, OWN], f32, tag="of")
                nc.scalar.activation(of, wps, Identity,
                                     bias=bias["b2"][:, oc:oc + 1])
                nc.vector.tensor_add(of, of, x2f[oc])
                nc.sync.dma_start(out=out_d[P * oc:P * oc + P, :], in_=of)

        pool_close(h2_cm, x2_cm, wgtE_cm, xo_cm)

    return nc


def host_prep(inputs):
    """Build per-core input maps + gather metadata. Pure numpy."""
    x = np.asarray(inputs["x"], np.float32)
    ln1_w = np.asarray(inputs["ln1_w"], np.float32)
    ln1_b = np.asarray(inputs["ln1_b"], np.float32)
    ln2_w = np.asarray(inputs["ln2_w"], np.float32)
    ln2_b = np.asarray(inputs["ln2_b"], np.float32)

    def cat_heads(w):
        return np.ascontiguousarray(
            np.transpose(np.asarray(w, np.float32), (1, 0, 2)).reshape(D, D))

    wq_c, wk_c, wv_c = (cat_heads(inputs[k]) for k in ("Wq", "Wk", "Wv"))
    bq_f = np.asarray(inputs["bq"], np.float32).reshape(-1)
    bk_f = np.asarray(inputs["bk"], np.float32).reshape(-1)
    bv_f = np.asarray(inputs["bv"], np.float32).reshape(-1)
    Wp = np.asarray(inputs["Wp"], np.float32)
    bp = np.asarray(inputs["bp"], np.float32)
    W1 = np.asarray(inputs["W1"], np.float32)
    b1 = np.asarray(inputs["b1"], np.float32)
    W2 = np.asarray(inputs["W2"], np.float32)
    b2 = np.asarray(inputs["b2"], np.float32)

    sc = 1.0 / np.sqrt(HS)
    wq_eff = ((ln1_w[:, None] * wq_c) * sc).astype(BF16)
    bq_eff = ((ln1_b @ wq_c + bq_f) * sc).astype(np.float32)
    wk_eff = (ln1_w[:, None] * wk_c).astype(BF16)
    bk_eff = (ln1_b @ wk_c + bk_f).astype(np.float32)
    wv_eff = (ln1_w[:, None] * wv_c).astype(BF16)
    bv_eff = (ln1_b @ wv_c + bv_f).astype(np.float32)
    wp_eff = Wp.astype(BF16)
    w1_eff = (ln2_w[:, None] * W1).astype(BF16)
    b1_eff = (ln2_b @ W1 + b1).astype(np.float32)
    w2_eff = W2.astype(BF16)

    def chunked(v, n):
        return np.ascontiguousarray(v.reshape(n, P).T).astype(np.float32)

    shared = {
        "wq": wq_eff, "wk": wk_eff, "wv": wv_eff, "wp": wp_eff,
        "w1": w1_eff, "w2": w2_eff,
        "bq": chunked(bq_eff, ND), "bk": chunked(bk_eff, ND),
        "bv": chunked(bv_eff, ND), "bp": chunked(bp, ND),
        "b1": chunked(b1_eff, NFF), "b2": chunked(b2, ND),
    }

    in_maps, gathers = [], []
    for c in range(N_CORES):
        b, j = c // 4, c % 4
        qA, qB = QW * j, QW * (7 - j)
        xT = np.ascontiguousarray(x[b].T).astype(BF16)
        xo = np.ascontiguousarray(
            np.concatenate([x[b, qA:qA + QW].T, x[b, qB:qB + QW].T],
                           axis=1)).astype(np.float32)
        # masks[kc][:, h*512 + 0:256] = causal(kc, qA); [256:512] = (kc, qB)
        ks = np.arange(P)[:, None]
        qs = np.arange(QW)[None, :]
        masks = np.zeros((NKC1, P, 2 * QW), np.float32)
        for kc in range(NKC1):
            masks[kc, :, 0:QW] = (P * kc + ks) <= (qA + qs)
            masks[kc, :, QW:2 * QW] = (P * kc + ks) <= (qB + qs)
        m = dict(shared)
        m["xT"] = xT
        m["xo"] = xo
        m["masks"] = masks.astype(BF16)
        in_maps.append(m)
        gathers.append((b, qA, qB))
    return in_maps, gathers


def make_nc():
    from concourse import bacc

    nc = bacc.Bacc("TRN2")
    build(nc)
    nc.compile()
    return nc


def kernel(**inputs):
    from concourse.bass_utils import run_bass_kernel_spmd

    nc = make_nc()
    in_maps, gathers = host_prep(inputs)
    res = run_bass_kernel_spmd(nc, in_maps, list(range(N_CORES)))
    out = np.zeros((B, S, D), np.float32)
    for c, (b, qA, qB) in enumerate(gathers):
        oT = res.results[c]["outT"]
        out[b, qA:qA + QW] = oT[:, 0:QW].T
        out[b, qB:qB + QW] = oT[:, QW:2 * QW].T
    return out

